# revision 1
# baseline (speedup 1.0000x reference)
"""Trainium2 Bass kernel for nn_CrossEntropyLoss_22419729285187.

Computes  -sum_{matched, non-BG true rows} dot(y_true[i,1:], y_pred[rank_i]) / count
sharded over 8 NeuronCores.

Strategy (per sharding hint): the host performs the cheap key join
(encode + searchsorted + cumsum) and compacts to the m_eff matched
(true,pred) row pairs — the r-th matched true row pairs positionally
with y_pred_features[r], so only the true side needs a gather and BG
rows are zeroed in place. The matched pairs are row-sharded across the
8 cores; each core streams its [rows, 32]+[rows, 32]+mask shard
(~19.5MB, large contiguous DMA tiles on both HWDGE rings + SWDGE) and
runs one fused multiply-reduce (scalar_tensor_tensor) per tile on the
DVE, accumulating per-tile partial sums into columns reduced once at
the end. Per-core [128, 2] partials (num, count) are summed on the
host for the final -num/k.

Measured on trn2 x8: ~66-76us HW exec, rel err ~1.5e-6.
"""

import os
import sys

for _p in ("/opt/trn_rl_repo", "/root/.axon_site/_ro/trn_rl_repo"):
    if os.path.isdir(_p) and _p not in sys.path:
        sys.path.append(_p)

import numpy as np

N_CORES = 8

# Device-side tiling: rows are laid out [tile t][partition p][group g];
# each of the 128 partitions owns G consecutive rows per tile.
PARTS = 128
G = 64  # rows per partition per tile (main segment)

_compiled = {}
_last_results = None


def _encode(idx):
    idx = idx.astype(np.int64)
    return ((idx[:, 0] * 1024 + idx[:, 1]) * 1024 + idx[:, 2]) * 1024 + idx[:, 3]


def _build_program(segments, c_pred):
    """Build + schedule the SPMD Tile program for one core shard.

    segments: list of (n_tiles, G) — the shard's rows are laid out
    [tile][partition][group] per segment, concatenated. Using a small
    trailing segment keeps zero-padding minimal while the main segment
    uses large (1MB) DMA tiles.
    """
    from concourse import bacc
    import concourse.mybir as mybir
    from concourse.tile import TileContext

    f32 = mybir.dt.float32
    r_pad = sum(nt * PARTS * g for nt, g in segments)
    total_tiles = sum(nt for nt, _ in segments)

    nc = bacc.Bacc("TRN2", target_bir_lowering=False, debug=False,
                   num_devices=N_CORES)
    yt_d = nc.dram_tensor("yt", [r_pad, c_pred], f32, kind="ExternalInput")
    yp_d = nc.dram_tensor("ypal", [r_pad, c_pred], f32, kind="ExternalInput")
    ax_d = nc.dram_tensor("aux", [r_pad, 1], f32, kind="ExternalInput")
    out_d = nc.dram_tensor("partials", [PARTS, 2], f32, kind="ExternalOutput")

    ax_w = r_pad // PARTS
    with TileContext(nc) as tc:
        with tc.tile_pool(name="acc", bufs=1) as accp:
            red_all = accp.tile([PARTS, total_tiles], f32)
            num_acc = accp.tile([PARTS, 1], f32)
            k_acc = accp.tile([PARTS, 1], f32)
            # k: row order is irrelevant for a global count — one flat
            # [128, r_pad/128] load + one fused square-reduce.
            ax_t = accp.tile([PARTS, ax_w], f32)
            kscr = accp.tile([PARTS, ax_w], f32)
            ax_flat = ax_d.ap().rearrange("(p w) c -> p (w c)", p=PARTS)
            nc.gpsimd.dma_start(out=ax_t[:], in_=ax_flat)
            nc.vector.scalar_tensor_tensor(
                out=kscr[:], in0=ax_t[:], scalar=1.0, in1=ax_t[:],
                op0=mybir.AluOpType.mult, op1=mybir.AluOpType.mult,
                accum_out=k_acc[:])
            with tc.tile_pool(name="io", bufs=5) as pool, \
                 tc.tile_pool(name="scrp", bufs=2) as scrp:
                row0 = 0
                ti = 0
                for nt, g in segments:
                    seg_rows = nt * PARTS * g
                    yt_v = yt_d.ap()[row0:row0 + seg_rows, :].rearrange(
                        "(t p g) c -> t p (g c)", p=PARTS, g=g)
                    yp_v = yp_d.ap()[row0:row0 + seg_rows, :].rearrange(
                        "(t p g) c -> t p (g c)", p=PARTS, g=g)
                    row0 += seg_rows
                    for t in range(nt):
                        yt_t = pool.tile([PARTS, g * c_pred], f32, tag="yt")
                        yp_t = pool.tile([PARTS, g * c_pred], f32, tag="yp")
                        nc.sync.dma_start(out=yt_t[:], in_=yt_v[t])
                        nc.scalar.dma_start(out=yp_t[:], in_=yp_v[t])
                        scr = scrp.tile([PARTS, g * c_pred], f32, tag="scr")
                        # red_all[:, ti] = sum_{g,c} yt * ypal
                        nc.vector.scalar_tensor_tensor(
                            out=scr[:], in0=yt_t[:], scalar=1.0, in1=yp_t[:],
                            op0=mybir.AluOpType.mult, op1=mybir.AluOpType.mult,
                            accum_out=red_all[:, ti:ti + 1])
                        ti += 1
            nc.vector.tensor_reduce(out=num_acc[:], in_=red_all[:],
                                    axis=mybir.AxisListType.X,
                                    op=mybir.AluOpType.add)
            nc.sync.dma_start(out=out_d[:, 0:1], in_=num_acc[:])
            nc.sync.dma_start(out=out_d[:, 1:2], in_=k_acc[:])
    nc.compile()
    return nc


def kernel(y_true_features, y_true_indices, y_pred_features, y_pred_indices):
    global _last_results
    from concourse.bass_utils import run_bass_kernel_spmd

    yt = np.ascontiguousarray(np.asarray(y_true_features, dtype=np.float32))
    yp = np.ascontiguousarray(np.asarray(y_pred_features, dtype=np.float32))
    n, c1 = yt.shape
    m, c = yp.shape

    # ---- host-side key join (cheap integer work) ----
    kt = _encode(np.asarray(y_true_indices))
    kp = _encode(np.asarray(y_pred_indices))
    kps = np.sort(kp)
    pos = np.clip(np.searchsorted(kps, kt), 0, m - 1)
    matched = kps[pos] == kt
    # Only matched true rows contribute to num and k. The r-th matched
    # true row (row order) pairs with y_pred_features[r] positionally
    # (rank = cumsum(matched)-1 is sequential over matched rows), so the
    # pred side needs no gather at all — just the first m_eff rows.
    midx = np.flatnonzero(matched)
    m_eff = midx.size
    yt_cmp = yt[midx, 1:]                      # [m_eff, c] gather
    notbg = yt[midx, 0] != 1.0
    yt_cmp[~notbg] = 0.0                       # BG pairs contribute 0
    aux = notbg.astype(np.float32)

    # ---- shard the m_eff matched pairs across cores ----
    rows = -(-m_eff // N_CORES)
    big = PARTS * G
    nt1 = rows // big
    rem = rows - nt1 * big
    g2 = -(-rem // PARTS)
    segments = ((nt1, G), (1, g2)) if g2 > 0 else ((nt1, G),)
    r_pad = sum(nt * PARTS * g for nt, g in segments)

    key = (segments, c)
    if key not in _compiled:
        _compiled[key] = _build_program(segments, c)
    nc = _compiled[key]

    in_maps = []
    for i in range(N_CORES):
        lo, hi = i * rows, min((i + 1) * rows, m_eff)
        nr = max(hi - lo, 0)
        yt_c = np.zeros((r_pad, c), dtype=np.float32)
        yt_c[:nr] = yt_cmp[lo:hi]
        yp_c = np.zeros((r_pad, c), dtype=np.float32)
        yp_c[:nr] = yp[lo:hi]
        ax_c = np.zeros((r_pad, 1), dtype=np.float32)
        ax_c[:nr, 0] = aux[lo:hi]
        in_maps.append({"yt": yt_c, "ypal": yp_c, "aux": ax_c})

    res = run_bass_kernel_spmd(nc, in_maps, list(range(N_CORES)))
    _last_results = res

    num = 0.0
    k = 0.0
    for i in range(N_CORES):
        p = res.results[i]["partials"]
        num += float(p[:, 0].sum(dtype=np.float64))
        k += float(p[:, 1].sum(dtype=np.float64))
    return np.float32(-num / k)



# revision 2
# speedup vs baseline: 1.7480x; 1.7480x over previous
"""Trainium2 Bass kernel for nn_CrossEntropyLoss_22419729285187.

Computes  -sum_{matched, non-BG true rows} dot(y_true[i,1:], y_pred[rank_i]) / count
sharded over 8 NeuronCores.

Strategy (per sharding hint): the host performs the cheap key join
(encode + searchsorted + cumsum) and compacts to the contributing
(matched AND non-background) row pairs — the r-th matched true row
pairs positionally with y_pred_features[r], so both sides are plain
host gathers.  The pairs are cast to fp16 (the f32->fp16 quantization
perturbs the result by ~3e-4 relative, far under the 2e-2 gate) and
row-sharded across the 8 cores; each core streams its two [rows, 32]
fp16 shards (~8.2MB total, 8KB-per-partition contiguous DMA lines on
both HWDGE rings) and runs one fused multiply-reduce
(scalar_tensor_tensor, 2-elem/cycle 16-bit DVE mode) per tile,
accumulating per-tile partial sums in fp32.  The non-BG count k and
the final -num/k division are host-side scalar work.
"""

import os
import sys

for _p in ("/opt/trn_rl_repo", "/root/.axon_site/_ro/trn_rl_repo"):
    if os.path.isdir(_p) and _p not in sys.path:
        sys.path.append(_p)

import numpy as np

N_CORES = 8

# Device-side tiling: rows are laid out [tile t][partition p][group g];
# each of the 128 partitions owns G consecutive rows per tile.
PARTS = 128
G = 128  # rows per partition per tile (main segment) -> 8KB fp16 lines

_compiled = {}
_last_results = None


def _encode(idx):
    idx = idx.astype(np.int64)
    return ((idx[:, 0] * 1024 + idx[:, 1]) * 1024 + idx[:, 2]) * 1024 + idx[:, 3]


def _build_program(segments, c_pred):
    """Build + schedule the SPMD Tile program for one core shard.

    segments: list of (n_tiles, G) — the shard's rows are laid out
    [tile][partition][group] per segment, concatenated. Using a small
    trailing segment keeps zero-padding minimal while the main segment
    uses large (1MB) DMA tiles.
    """
    from concourse import bacc
    import concourse.mybir as mybir
    from concourse.tile import TileContext

    f16 = mybir.dt.float16
    f32 = mybir.dt.float32
    r_pad = sum(nt * PARTS * g for nt, g in segments)
    total_tiles = sum(nt for nt, _ in segments)

    nc = bacc.Bacc("TRN2", target_bir_lowering=False, debug=False,
                   num_devices=N_CORES)
    yt_d = nc.dram_tensor("yt", [r_pad, c_pred], f16, kind="ExternalInput")
    yp_d = nc.dram_tensor("ypal", [r_pad, c_pred], f16, kind="ExternalInput")
    out_d = nc.dram_tensor("partials", [PARTS, total_tiles], f32,
                           kind="ExternalOutput")

    with TileContext(nc) as tc:
        with tc.tile_pool(name="acc", bufs=1) as accp:
            red_all = accp.tile([PARTS, total_tiles], f32)
            with tc.tile_pool(name="io", bufs=4) as pool, \
                 tc.tile_pool(name="scrp", bufs=2) as scrp:
                row0 = 0
                ti = 0
                for nt, g in segments:
                    seg_rows = nt * PARTS * g
                    yt_v = yt_d.ap()[row0:row0 + seg_rows, :].rearrange(
                        "(t p g) c -> t p (g c)", p=PARTS, g=g)
                    yp_v = yp_d.ap()[row0:row0 + seg_rows, :].rearrange(
                        "(t p g) c -> t p (g c)", p=PARTS, g=g)
                    row0 += seg_rows
                    for t in range(nt):
                        yt_t = pool.tile([PARTS, g * c_pred], f16, tag="yt")
                        yp_t = pool.tile([PARTS, g * c_pred], f16, tag="yp")
                        nc.sync.dma_start(out=yt_t[:], in_=yt_v[t])
                        nc.scalar.dma_start(out=yp_t[:], in_=yp_v[t])
                        scr = scrp.tile([PARTS, g * c_pred], f16, tag="scr")
                        # red_all[:, ti] = sum_{g,c} yt * ypal
                        nc.vector.scalar_tensor_tensor(
                            out=scr[:], in0=yt_t[:], scalar=1.0, in1=yp_t[:],
                            op0=mybir.AluOpType.mult, op1=mybir.AluOpType.mult,
                            accum_out=red_all[:, ti:ti + 1])
                        ti += 1
            nc.sync.dma_start(out=out_d[:], in_=red_all[:])
    nc.compile()
    return nc


def kernel(y_true_features, y_true_indices, y_pred_features, y_pred_indices):
    global _last_results
    from concourse.bass_utils import run_bass_kernel_spmd

    yt = np.asarray(y_true_features, dtype=np.float32)
    yp = np.asarray(y_pred_features, dtype=np.float32)
    n, c1 = yt.shape
    m, c = yp.shape

    # ---- host-side key join (cheap integer work) ----
    kt = _encode(np.asarray(y_true_indices))
    kp = _encode(np.asarray(y_pred_indices))
    kps = np.sort(kp)
    pos = np.clip(np.searchsorted(kps, kt), 0, m - 1)
    matched = kps[pos] == kt
    # Only matched, non-background true rows contribute. The r-th
    # matched true row (row order) pairs with y_pred_features[r]
    # positionally (rank = cumsum(matched)-1 is sequential over matched
    # rows), so compacting to the contributing pairs is two host
    # gathers; k is their count.
    midx = np.flatnonzero(matched)
    keep = np.flatnonzero(yt[midx, 0] != 1.0)   # positions within matched
    k = keep.size
    yt_cmp = yt[midx[keep], 1:].astype(np.float16)   # [k, c]
    yp_cmp = yp[keep].astype(np.float16)             # [k, c]

    # ---- shard the k contributing pairs across cores ----
    rows = -(-k // N_CORES)
    big = PARTS * G
    nt1 = rows // big
    rem = rows - nt1 * big
    g2 = -(-rem // PARTS)
    segments = ((nt1, G), (1, g2)) if g2 > 0 else ((nt1, G),)
    r_pad = sum(nt * PARTS * g for nt, g in segments)

    key = (segments, c)
    if key not in _compiled:
        _compiled[key] = _build_program(segments, c)
    nc = _compiled[key]

    in_maps = []
    for i in range(N_CORES):
        lo, hi = i * rows, min((i + 1) * rows, k)
        nr = max(hi - lo, 0)
        yt_c = np.zeros((r_pad, c), dtype=np.float16)
        yt_c[:nr] = yt_cmp[lo:hi]
        yp_c = np.zeros((r_pad, c), dtype=np.float16)
        yp_c[:nr] = yp_cmp[lo:hi]
        in_maps.append({"yt": yt_c, "ypal": yp_c})

    res = run_bass_kernel_spmd(nc, in_maps, list(range(N_CORES)))
    _last_results = res

    num = 0.0
    for i in range(N_CORES):
        num += float(res.results[i]["partials"].sum(dtype=np.float64))
    return np.float32(-num / k)


# revision 5
# speedup vs baseline: 1.8939x; 1.0834x over previous
"""Trainium2 Bass kernel for nn_CrossEntropyLoss_22419729285187.

Computes  -sum_{matched, non-BG true rows} dot(y_true[i,1:], y_pred[rank_i]) / count
sharded over 8 NeuronCores.

Strategy (per sharding hint): the host performs the cheap key join
(encode + searchsorted + cumsum) and compacts to the contributing
(matched AND non-background) row pairs — the r-th matched true row
pairs positionally with y_pred_features[r], so both sides are plain
host gathers.  The pairs are cast to fp16 (the f32->fp16 quantization
perturbs the result by ~3e-4 relative, far under the 2e-2 gate) and
row-sharded across the 8 cores; each core streams its two [rows, 32]
fp16 shards (~8.2MB total, 8KB-per-partition contiguous DMA lines on
both HWDGE rings) and runs one fused multiply-reduce
(scalar_tensor_tensor, 2-elem/cycle 16-bit DVE mode) per tile,
accumulating per-tile partial sums in fp32.  The non-BG count k and
the final -num/k division are host-side scalar work.
"""

import os
import sys

for _p in ("/opt/trn_rl_repo", "/root/.axon_site/_ro/trn_rl_repo"):
    if os.path.isdir(_p) and _p not in sys.path:
        sys.path.append(_p)

import numpy as np

N_CORES = 8

# Device-side tiling: rows are laid out [tile t][partition p][group g];
# each of the 128 partitions owns G consecutive rows per tile.
PARTS = 128
G = 64  # rows per partition per tile (main segment) -> 4KB fp16 lines

_compiled = {}
_last_results = None


def _encode(idx):
    idx = idx.astype(np.int64)
    return ((idx[:, 0] * 1024 + idx[:, 1]) * 1024 + idx[:, 2]) * 1024 + idx[:, 3]


def _build_program(segments, c_pred):
    """Build + schedule the SPMD Tile program for one core shard.

    segments: list of (n_tiles, G) — the shard's rows are laid out
    [tile][partition][group] per segment, concatenated. Using a small
    trailing segment keeps zero-padding minimal while the main segment
    uses large (1MB) DMA tiles.
    """
    from concourse import bacc
    import concourse.mybir as mybir
    from concourse.tile import TileContext

    f16 = mybir.dt.float16
    f32 = mybir.dt.float32
    r_pad = sum(nt * PARTS * g for nt, g in segments)
    total_tiles = sum(nt for nt, _ in segments)

    nc = bacc.Bacc("TRN2", target_bir_lowering=False, debug=False,
                   num_devices=N_CORES)
    yt_d = nc.dram_tensor("yt", [r_pad, c_pred], f16, kind="ExternalInput")
    yp_d = nc.dram_tensor("ypal", [r_pad, c_pred], f16, kind="ExternalInput")
    out_d = nc.dram_tensor("partials", [PARTS, total_tiles], f32,
                           kind="ExternalOutput")

    with TileContext(nc) as tc:
        with tc.tile_pool(name="acc", bufs=1) as accp:
            red_all = accp.tile([PARTS, total_tiles], f32)
            # One distinct buffer per tile: every DMA can be issued
            # upfront with no buffer-reuse stalls (fits SBUF: ~16 tiles
            # x 2 tensors x 4KB/partition ~ 128KB of 208KB).
            with tc.tile_pool(name="io", bufs=total_tiles) as pool, \
                 tc.tile_pool(name="scrp", bufs=2) as scrp:
                row0 = 0
                ti = 0
                for nt, g in segments:
                    seg_rows = nt * PARTS * g
                    yt_v = yt_d.ap()[row0:row0 + seg_rows, :].rearrange(
                        "(t p g) c -> t p (g c)", p=PARTS, g=g)
                    yp_v = yp_d.ap()[row0:row0 + seg_rows, :].rearrange(
                        "(t p g) c -> t p (g c)", p=PARTS, g=g)
                    row0 += seg_rows
                    for t in range(nt):
                        yt_t = pool.tile([PARTS, g * c_pred], f16, tag="yt")
                        yp_t = pool.tile([PARTS, g * c_pred], f16, tag="yp")
                        nc.sync.dma_start(out=yt_t[:], in_=yt_v[t])
                        nc.scalar.dma_start(out=yp_t[:], in_=yp_v[t])
                        scr = scrp.tile([PARTS, g * c_pred], f16, tag="scr")
                        # red_all[:, ti] = sum_{g,c} yt * ypal
                        nc.vector.scalar_tensor_tensor(
                            out=scr[:], in0=yt_t[:], scalar=1.0, in1=yp_t[:],
                            op0=mybir.AluOpType.mult, op1=mybir.AluOpType.mult,
                            accum_out=red_all[:, ti:ti + 1])
                        ti += 1
            nc.sync.dma_start(out=out_d[:], in_=red_all[:])
    nc.compile()
    return nc


def kernel(y_true_features, y_true_indices, y_pred_features, y_pred_indices):
    global _last_results
    from concourse.bass_utils import run_bass_kernel_spmd

    yt = np.asarray(y_true_features, dtype=np.float32)
    yp = np.asarray(y_pred_features, dtype=np.float32)
    n, c1 = yt.shape
    m, c = yp.shape

    # ---- host-side key join (cheap integer work) ----
    kt = _encode(np.asarray(y_true_indices))
    kp = _encode(np.asarray(y_pred_indices))
    kps = np.sort(kp)
    pos = np.clip(np.searchsorted(kps, kt), 0, m - 1)
    matched = kps[pos] == kt
    # Only matched, non-background true rows contribute. The r-th
    # matched true row (row order) pairs with y_pred_features[r]
    # positionally (rank = cumsum(matched)-1 is sequential over matched
    # rows), so compacting to the contributing pairs is two host
    # gathers; k is their count.
    midx = np.flatnonzero(matched)
    keep = np.flatnonzero(yt[midx, 0] != 1.0)   # positions within matched
    k = keep.size
    yt_cmp = yt[midx[keep], 1:].astype(np.float16)   # [k, c]
    yp_cmp = yp[keep].astype(np.float16)             # [k, c]

    # ---- shard the k contributing pairs across cores ----
    rows = -(-k // N_CORES)
    units = -(-rows // PARTS)           # 128-row units per core
    nbig = units // G
    rem = units - nbig * G
    # Keep the final tile tiny (8 units) so the unavoidable post-stream
    # DVE step after the last byte lands is ~0.3us, not ~4us.
    segs = []
    if rem > 8:
        segs = [(nbig, G), (1, rem - 8), (1, 8)]
    elif rem > 0:
        segs = [(nbig, G), (1, rem)]
    elif nbig > 0:
        segs = [(nbig - 1, G), (1, G - 8), (1, 8)] if nbig > 1 else [(1, G)]
    segments = tuple((nt, g) for nt, g in segs if nt > 0 and g > 0)
    r_pad = sum(nt * PARTS * g for nt, g in segments)

    key = (segments, c)
    if key not in _compiled:
        _compiled[key] = _build_program(segments, c)
    nc = _compiled[key]

    in_maps = []
    for i in range(N_CORES):
        lo, hi = i * rows, min((i + 1) * rows, k)
        nr = max(hi - lo, 0)
        yt_c = np.zeros((r_pad, c), dtype=np.float16)
        yt_c[:nr] = yt_cmp[lo:hi]
        yp_c = np.zeros((r_pad, c), dtype=np.float16)
        yp_c[:nr] = yp_cmp[lo:hi]
        in_maps.append({"yt": yt_c, "ypal": yp_c})

    res = run_bass_kernel_spmd(nc, in_maps, list(range(N_CORES)))
    _last_results = res

    num = 0.0
    for i in range(N_CORES):
        num += float(res.results[i]["partials"].sum(dtype=np.float64))
    return np.float32(-num / k)


# revision 6
# speedup vs baseline: 1.9119x; 1.0095x over previous
"""Trainium2 Bass kernel for nn_CrossEntropyLoss_22419729285187.

Computes  -sum_{matched, non-BG true rows} dot(y_true[i,1:], y_pred[rank_i]) / count
sharded over 8 NeuronCores.

Strategy (per sharding hint): the host performs the cheap key join
(encode + searchsorted + cumsum) and compacts to the contributing
(matched AND non-background) row pairs — the r-th matched true row
pairs positionally with y_pred_features[r], so both sides are plain
host gathers.  The pairs are cast to fp16 (~3e-4 relative
perturbation, far under the 2e-2 gate) and row-sharded across the 8
cores.  Per core the two shards are interleaved into ONE stream
([tile][partition][yt-line | yp-line]) so each tile is a single DMA
with 8KB contiguous per-partition lines — fewer descriptors and
completion-semaphore lanes, so tiles arrive in strict FIFO order right
behind the bytes.  The DVE runs one fused multiply-reduce
(scalar_tensor_tensor) per tile, accumulating per-tile partial sums in
fp32; a descending mini-tail keeps the post-stream DVE work ~0.3us.
The non-BG count k and the final -num/k division are host-side scalar
work.
"""

import os
import sys

for _p in ("/opt/trn_rl_repo", "/root/.axon_site/_ro/trn_rl_repo"):
    if os.path.isdir(_p) and _p not in sys.path:
        sys.path.append(_p)

import numpy as np

N_CORES = 8

PARTS = 128
G = 64  # rows per partition per tile (main segment)

_compiled = {}
_last_results = None


def _encode(idx):
    idx = idx.astype(np.int64)
    return ((idx[:, 0] * 1024 + idx[:, 1]) * 1024 + idx[:, 2]) * 1024 + idx[:, 3]


def _plan_segments(rows):
    """Tile plan for one core: list of g (rows-per-partition) per tile."""
    units = -(-rows // PARTS)  # 128-row units
    nbig = units // G
    rem = units - nbig * G
    gs = [G] * nbig
    if rem > 8:
        gs += [rem - 8, 8]
    elif rem > 0:
        gs += [rem]
    elif nbig > 1:
        gs = [G] * (nbig - 1) + [G - 8, 8]
    return gs


def _build_program(gs, c_pred):
    """Build + schedule the SPMD Tile program for one core shard.

    gs: rows-per-partition for each tile. The single input stream is
    laid out [tile][partition][g*c yt | g*c yp] fp16, contiguous.
    """
    from concourse import bacc
    import concourse.mybir as mybir
    from concourse.tile import TileContext

    f16 = mybir.dt.float16
    f32 = mybir.dt.float32
    total = sum(2 * g * c_pred * PARTS for g in gs)
    n_tiles = len(gs)

    nc = bacc.Bacc("TRN2", target_bir_lowering=False, debug=False,
                   num_devices=N_CORES)
    x_d = nc.dram_tensor("x", [total], f16, kind="ExternalInput")
    out_d = nc.dram_tensor("partials", [PARTS, n_tiles], f32,
                           kind="ExternalOutput")

    with TileContext(nc) as tc:
        with tc.tile_pool(name="acc", bufs=1) as accp:
            red_all = accp.tile([PARTS, n_tiles], f32)
            # One distinct buffer per tile: all DMAs are issued upfront
            # and arrive in FIFO order with no buffer-reuse stalls.
            with tc.tile_pool(name="io", bufs=n_tiles) as pool, \
                 tc.tile_pool(name="scrp", bufs=2) as scrp:
                off = 0
                for ti, g in enumerate(gs):
                    w = 2 * g * c_pred
                    view = x_d.ap()[off:off + w * PARTS].rearrange(
                        "(p w) -> p w", p=PARTS)
                    off += w * PARTS
                    xt = pool.tile([PARTS, w], f16, tag="x")
                    nc.sync.dma_start(out=xt[:], in_=view)
                    scr = scrp.tile([PARTS, g * c_pred], f16, tag="scr")
                    # red_all[:, ti] = sum_{g,c} yt * yp
                    nc.vector.scalar_tensor_tensor(
                        out=scr[:], in0=xt[:, :g * c_pred], scalar=1.0,
                        in1=xt[:, g * c_pred:],
                        op0=mybir.AluOpType.mult, op1=mybir.AluOpType.mult,
                        accum_out=red_all[:, ti:ti + 1])
            nc.scalar.dma_start(out=out_d[:], in_=red_all[:])
    nc.compile()
    return nc


def kernel(y_true_features, y_true_indices, y_pred_features, y_pred_indices):
    global _last_results
    from concourse.bass_utils import run_bass_kernel_spmd

    yt = np.asarray(y_true_features, dtype=np.float32)
    yp = np.asarray(y_pred_features, dtype=np.float32)
    n, c1 = yt.shape
    m, c = yp.shape

    # ---- host-side key join (cheap integer work) ----
    kt = _encode(np.asarray(y_true_indices))
    kp = _encode(np.asarray(y_pred_indices))
    kps = np.sort(kp)
    pos = np.clip(np.searchsorted(kps, kt), 0, m - 1)
    matched = kps[pos] == kt
    # Only matched, non-background true rows contribute. The r-th
    # matched true row (row order) pairs with y_pred_features[r]
    # positionally (rank = cumsum(matched)-1 is sequential over matched
    # rows), so compacting to the contributing pairs is two host
    # gathers; k is their count.
    midx = np.flatnonzero(matched)
    keep = np.flatnonzero(yt[midx, 0] != 1.0)   # positions within matched
    k = keep.size
    yt_cmp = yt[midx[keep], 1:].astype(np.float16)   # [k, c]
    yp_cmp = yp[keep].astype(np.float16)             # [k, c]

    # ---- shard the k contributing pairs across cores ----
    rows = -(-k // N_CORES)
    gs = _plan_segments(rows)
    r_pad = PARTS * sum(gs)

    key = (tuple(gs), c)
    if key not in _compiled:
        _compiled[key] = _build_program(gs, c)
    nc = _compiled[key]

    total = 2 * r_pad * c
    in_maps = []
    for i in range(N_CORES):
        lo, hi = i * rows, min((i + 1) * rows, k)
        nr = max(hi - lo, 0)
        a = np.zeros((r_pad, c), dtype=np.float16)
        a[:nr] = yt_cmp[lo:hi]
        b = np.zeros((r_pad, c), dtype=np.float16)
        b[:nr] = yp_cmp[lo:hi]
        # interleave per tile: [tile][partition][g*c of a | g*c of b]
        x = np.empty(total, dtype=np.float16)
        off = 0
        r0 = 0
        for g in gs:
            trows = PARTS * g
            w = 2 * g * c
            blk = x[off:off + trows * 2 * c].reshape(PARTS, w)
            blk[:, :g * c] = a[r0:r0 + trows].reshape(PARTS, g * c)
            blk[:, g * c:] = b[r0:r0 + trows].reshape(PARTS, g * c)
            off += trows * 2 * c
            r0 += trows
        in_maps.append({"x": x})

    res = run_bass_kernel_spmd(nc, in_maps, list(range(N_CORES)))
    _last_results = res

    num = 0.0
    for i in range(N_CORES):
        num += float(res.results[i]["partials"].sum(dtype=np.float64))
    return np.float32(-num / k)
